# revision 9
# baseline (speedup 1.0000x reference)
"""Multi-head causal attention (bs=4, L=2048, d_model=512, 8 heads x 64) on 8
Trainium2 NeuronCores.

Sharding: core c = (batch b = c//2, head-group hg = c%2); each core computes 4
heads of one batch over the full sequence. Host pre-transposes activations and
weight slices so every device matmul has its contraction dim on partitions;
device returns the transposed partial output projection; host sums the two
head-group partials per batch, transposes back and adds the (folded) biases.
"""

import numpy as np

import concourse.bacc as bacc
import concourse.mybir as mybir
import concourse.tile as tile
from concourse.bass_utils import run_bass_kernel_spmd

F32 = mybir.dt.float32
F32R = mybir.dt.float32r
AF = mybir.ActivationFunctionType

L = 2048          # sequence length
D = 512           # model dim
HD = 256          # head-group output dim (4 heads x 64)
DK = 64           # head dim
NH = 4            # heads per core
P = 128
IB = 512          # query block (i) width
NIB = L // IB     # 4 query blocks
NKT = D // P      # 4 contraction tiles over model dim
NJT = L // P      # 16 key tiles
SCALE = 1.0 / 8.0  # 1/sqrt(DK)

GRP = 2           # score j-tiles per PSUM/exp group


def _build(use_f32r=True):
    nc = bacc.Bacc("TRN2", target_bir_lowering=False, debug=False,
                   enable_asserts=False)

    xT = nc.dram_tensor("xT", [D, L], F32R, kind="ExternalInput")
    wq = nc.dram_tensor("wq", [D, HD], F32R, kind="ExternalInput")
    wk = nc.dram_tensor("wk", [D, HD], F32R, kind="ExternalInput")
    wv = nc.dram_tensor("wv", [D, HD], F32R, kind="ExternalInput")
    wo = nc.dram_tensor("wo", [HD, D], F32R, kind="ExternalInput")
    bq = nc.dram_tensor("bq", [HD], F32, kind="ExternalInput")
    bk = nc.dram_tensor("bk", [HD], F32, kind="ExternalInput")
    outT = nc.dram_tensor("outT", [D, L], F32, kind="ExternalOutput")

    def r(ap):
        return ap

    with tile.TileContext(nc) as tc:
        with (
            tc.tile_pool(name="w", bufs=1) as pool_w,
            tc.tile_pool(name="x", bufs=NKT) as pool_x,
            tc.tile_pool(name="qk", bufs=1) as pool_qk,
            tc.tile_pool(name="v", bufs=NJT) as pool_v,
            tc.tile_pool(name="at", bufs=3) as pool_at,
            tc.tile_pool(name="zc", bufs=2) as pool_zc,
            tc.tile_pool(name="nm", bufs=2) as pool_nm,
            tc.tile_pool(name="o", bufs=2) as pool_o,
            tc.tile_pool(name="ps", bufs=2, space="PSUM") as pool_ps,
            tc.tile_pool(name="pz", bufs=2, space="PSUM") as pool_pz,
            tc.tile_pool(name="pp", bufs=2, space="PSUM") as pool_pp,
        ):
            # ---- loads ----
            wq_sb = pool_w.tile([P, NKT, HD], F32R, tag="wq")
            wk_sb = pool_w.tile([P, NKT, HD], F32R, tag="wk")
            wv_sb = pool_w.tile([P, NKT, HD], F32R, tag="wv")
            wo_sb = pool_w.tile([P, HD // P, D], F32R, tag="wo")
            bq_sb = pool_w.tile([P, HD // P], F32, tag="bq")
            bk_sb = pool_w.tile([P, HD // P], F32, tag="bk")
            nc.sync.dma_start(wq_sb[:], wq.ap().rearrange("(t p) n -> p t n", p=P))
            nc.sync.dma_start(wk_sb[:], wk.ap().rearrange("(t p) n -> p t n", p=P))
            nc.sync.dma_start(wv_sb[:], wv.ap().rearrange("(t p) n -> p t n", p=P))
            nc.sync.dma_start(wo_sb[:], wo.ap().rearrange("(t p) n -> p t n", p=P))
            nc.sync.dma_start(bq_sb[:], bq.ap().rearrange("(t p) -> p t", p=P))
            nc.sync.dma_start(bk_sb[:], bk.ap().rearrange("(t p) -> p t", p=P))

            xts = []
            for kt in range(NKT):
                xt = pool_x.tile([P, L], F32R)
                nc.sync.dma_start(xt[:], xT.ap()[kt * P:(kt + 1) * P, :])
                xts.append(xt)

            # ---- q/k projections: qT/kT[d, i] per d-tile (2 heads each) ----
            qk_tiles = {}
            for name, w_sb, b_sb in (("q", wq_sb, bq_sb), ("k", wk_sb, bk_sb)):
                for dt in range(HD // P):
                    dst = pool_qk.tile([P, L], F32R, tag=f"{name}{dt}")
                    qk_tiles[(name, dt)] = dst
                    for ic in range(NIB):
                        pp = pool_pp.tile([P, IB], F32, tag="pp")
                        for kt in range(NKT):
                            nc.tensor.matmul(
                                pp[:],
                                lhsT=r(w_sb[:, kt, dt * P:(dt + 1) * P]),
                                rhs=r(xts[kt][:, ic * IB:(ic + 1) * IB]),
                                start=(kt == 0), stop=(kt == NKT - 1),
                            )
                        nc.vector.tensor_scalar_add(
                            dst[:, ic * IB:(ic + 1) * IB], pp[:],
                            b_sb[:, dt:dt + 1])

            # ---- v projection: natural layout [j, (h, 65)], col 64 == 1.0 ----
            vts = []
            for jt in range(NJT):
                vt = pool_v.tile([P, NH, DK + 1], F32R, tag="v")
                pp = pool_pp.tile([P, HD], F32, tag="pp")
                for kt in range(NKT):
                    nc.tensor.matmul(
                        pp[:],
                        lhsT=r(xts[kt][:, jt * P:(jt + 1) * P]),
                        rhs=r(wv_sb[:, kt, :]),
                        start=(kt == 0), stop=(kt == NKT - 1),
                    )
                nc.vector.tensor_copy(
                    vt[:, :, 0:DK], pp[:].rearrange("p (h e) -> p h e", h=NH))
                nc.vector.memset(vt[:, :, DK:DK + 1].bitcast(F32), 1.0)
                vts.append(vt)

            # ---- attention + output projection, per query block ----
            for ib in range(NIB):
                zc = [pool_zc.tile([P, IB], F32R, tag=f"zc{dt}", name=f"zc{dt}")
                      for dt in range(HD // P)]
                for h in range(NH):
                    dt, par = h // 2, h % 2
                    qt = qk_tiles[("q", dt)]
                    kt_t = qk_tiles[("k", dt)]
                    drow = par * DK  # 0 or 64: base partition of this head
                    nj = 4 * (ib + 1)
                    psz = pool_pz.tile([P, IB], F32, tag="pz")
                    vcol = h * (DK + 1)  # [v_h | ones] -> z rows 0:64, den 64
                    zrow = 0
                    for g in range((nj + GRP - 1) // GRP):
                        jbs = [g * GRP + s for s in range(GRP)
                               if g * GRP + s < nj]
                        ps = pool_ps.tile([P, GRP, IB], F32, tag="ps")
                        at = pool_at.tile([P, GRP, IB], F32R, tag="at")
                        for s, jb in enumerate(jbs):
                            nc.tensor.matmul(
                                ps[:, s, :],
                                lhsT=r(kt_t[drow:drow + DK,
                                            jb * P:(jb + 1) * P]),
                                rhs=r(qt[drow:drow + DK,
                                         ib * IB:(ib + 1) * IB]),
                                start=True, stop=True,
                            )
                        nc.scalar.activation(at[:], ps[:], AF.Exp, scale=SCALE)
                        # causal mask on diagonal groups: keep iff
                        # i - j - 128*(t0+s) >= 0
                        t0 = g * GRP - 4 * ib
                        if t0 + GRP > 0:
                            nc.gpsimd.affine_select(
                                at[:], at[:],
                                pattern=[[-P, GRP], [1, IB]],
                                compare_op=mybir.AluOpType.is_ge,
                                fill=0.0, base=-P * t0, channel_multiplier=-1,
                            )
                        for s, jb in enumerate(jbs):
                            vflat = vts[jb][:].rearrange("p h e -> p (h e)")
                            nc.tensor.matmul(
                                psz[zrow:zrow + DK + 1, :],
                                lhsT=r(vflat[:, vcol:vcol + DK + 1]),
                                rhs=r(at[:, s, :]),
                                start=(jb == 0), stop=(jb == nj - 1),
                            )
                    # normalize: z / denom (den row = partition 64)
                    rec = pool_nm.tile([P, IB], F32, tag="rec")
                    bct = pool_nm.tile([P, IB], F32, tag="bct")
                    nc.vector.reciprocal(rec[DK:DK + 1, :],
                                         psz[DK:DK + 1, :])
                    # partition_broadcast's Q7 cpu0 can only read partitions
                    # 0-15: bounce the reciprocal row to partition 0 first
                    nc.sync.dma_start(rec[0:1, :], rec[DK:DK + 1, :])
                    nc.gpsimd.partition_broadcast(
                        bct[0:DK, :], rec[0:1, :], channels=DK)
                    if par == 0:
                        nc.vector.tensor_mul(zc[dt][0:DK, :],
                                             psz[0:DK, :], bct[0:DK, :])
                    else:
                        # DVE lanes are partition-locked; shift the odd head's
                        # rows 0:64 -> 64:128 with an SBUF->SBUF DMA hop
                        zn = pool_nm.tile([P, IB], F32R, tag="zn")
                        nc.vector.tensor_mul(zn[0:DK, :],
                                             psz[0:DK, :], bct[0:DK, :])
                        nc.sync.dma_start(zc[dt][DK:P, :], zn[0:DK, :])

                # output projection for this query block
                for mt in range(D // P):
                    po = pool_pp.tile([P, IB], F32, tag="pp")
                    for kt2 in range(HD // P):
                        nc.tensor.matmul(
                            po[:],
                            lhsT=r(wo_sb[:, kt2, mt * P:(mt + 1) * P]),
                            rhs=r(zc[kt2][:]),
                            start=(kt2 == 0), stop=(kt2 == HD // P - 1),
                        )
                    osb = pool_o.tile([P, IB], F32, tag="o")
                    nc.vector.tensor_copy(osb[:], po[:])
                    nc.sync.dma_start(
                        outT.ap()[mt * P:(mt + 1) * P, ib * IB:(ib + 1) * IB],
                        osb[:])

    nc.compile()
    return nc


_NC = None


def _get_nc():
    global _NC
    if _NC is None:
        _NC = _build()
    return _NC


def _in_maps(x, w_q, b_q, w_k, b_k, w_v, b_v, w_o, b_o):
    maps = []
    for b in range(4):
        xTb = np.ascontiguousarray(x[b].T.astype(np.float32))
        for hg in range(2):
            sl = slice(hg * HD, (hg + 1) * HD)
            maps.append({
                "xT": xTb,
                "wq": np.ascontiguousarray(w_q[sl].T.astype(np.float32)),
                "wk": np.ascontiguousarray(w_k[sl].T.astype(np.float32)),
                "wv": np.ascontiguousarray(w_v[sl].T.astype(np.float32)),
                "wo": np.ascontiguousarray(w_o[:, sl].T.astype(np.float32)),
                "bq": np.ascontiguousarray(b_q[sl].astype(np.float32)),
                "bk": np.ascontiguousarray(b_k[sl].astype(np.float32)),
            })
    return maps


def _combine(results, w_o, b_v, b_o):
    corr = (b_o + w_o @ b_v).astype(np.float32)  # fold v/out biases
    out = np.empty((4, L, D), dtype=np.float32)
    for b in range(4):
        acc = results[2 * b]["outT"] + results[2 * b + 1]["outT"]
        out[b] = acc.T + corr
    return out


def kernel(x, w_q, b_q, w_k, b_k, w_v, b_v, w_o, b_o):
    nc = _get_nc()
    maps = _in_maps(x, w_q, b_q, w_k, b_k, w_v, b_v, w_o, b_o)
    res = run_bass_kernel_spmd(nc, maps, core_ids=list(range(8)))
    return _combine(res.results, w_o, b_v, b_o)


def bench(x, w_q, b_q, w_k, b_k, w_v, b_v, w_o, b_o):
    """Run with NTFF tracing; returns (output, exec_time_ns)."""
    nc = _get_nc()
    maps = _in_maps(x, w_q, b_q, w_k, b_k, w_v, b_v, w_o, b_o)
    res = run_bass_kernel_spmd(nc, maps, core_ids=list(range(8)), trace=True)
    return _combine(res.results, w_o, b_v, b_o), res.exec_time_ns
